# revision 10
# baseline (speedup 1.0000x reference)
"""Gaussian self-attention Trainium2 kernel (8-core data-parallel over batch).

Module: scores[i,j,h,k,l] = u_h . [dx, dy, dx^2, dy^2, dx*dy], dx=k-i, dy=l-j
        probs = softmax over (k,l); vals = probs @ hidden; out = vals @ W^T + b

Key structure: scores depend only on (dx, dy) in [-31,31]^2, so the softmax
numerator is a 63x63 table per head and the denominator Z a 32x32 box-sum.
The host precomputes (from the tiny parameter tensors) the exp tables and 1/Z;
the device materializes nothing: each core DMA-loads a per-partition shifted
strip S[p, u] = tab[63*(p//32) + (p%32) + lo_h + u] and the attention matmul
reads shifted windows of S directly as the moving operand:

  O^T[din, ij] = sum_kl X[kl, din] * U^T[kl, ij]        (stage A, PE bf16)
  rhs[p, (i,j)] = S[p, 1764 + 1008*n - 252*c - lo_h + 63*i + j]  (kl-chunk c)
  (partition p corresponds to kl = 128*c + 127 - p; X is pre-reversed to match)

  V = O^T * (1/Z[ij])                                    (DVE, during PSUM copy)
  out^T[dout, ij] = sum_{h,din} W^T[dout, (h,din)] V[(h,din), ij] + b (stage B)

The Gaussian tables are ~zero outside a small window, so (h, n, c) kl-chunks
whose dropped softmax mass is < 1e-4 everywhere are skipped entirely (~30%
of stage-A rows), and each head's strip is cropped to its live span.
Stage A interleaves 3 heads x 2 ij-halves per x chunk; stage B keeps W
stationary and streams V 512 wide (half the matmuls of the V-stationary
form). All PE operands bf16 (PSUM accumulates f32). Stage B emits out^T
([D, S] per batch); the host transposes.
"""
import sys
import types

import numpy as np


def _ensure_ntff_hook():
    """Install antenv.axon_hooks shim if the image lacks it (else NTFF
    tracing crashes run_bass_kernel_spmd under BASS_TRACE=1)."""
    try:
        import antenv.axon_hooks  # noqa: F401
        return
    except ImportError:
        pass
    try:
        import antenv
    except ImportError:
        antenv = types.ModuleType("antenv")
        sys.modules["antenv"] = antenv
    mod = types.ModuleType("antenv.axon_hooks")
    mod._hook = None
    mod.set_axon_ntff_profile_hook = lambda h: setattr(mod, "_hook", h)
    mod.get_axon_ntff_profile_hook = lambda: mod._hook
    sys.modules["antenv.axon_hooks"] = mod
    antenv.axon_hooks = mod
    try:
        from trn_agent_boot.trn_boot import _ntff_profile_via_ctypes
        h = _ntff_profile_via_ctypes("/opt/axon/libaxon_pjrt.so")
        if h is not None:
            mod._hook = h
    except Exception:
        pass


_ensure_ntff_hook()

import concourse.bacc as bacc
import concourse.bass as bass
import concourse.mybir as mybir
from concourse.tile import TileContext
from concourse.bass_utils import run_bass_kernel_spmd

B, W_IMG, H_IMG, D = 16, 32, 32, 256
NH = 9
S = W_IMG * H_IMG          # 1024 positions
NCORES = 8
BLOC = B // NCORES         # batches per core
TBL = 63 * 63              # 3969
F32 = mybir.dt.float32
BF16 = mybir.dt.bfloat16
SKIP_THR = 1e-4            # max dropped softmax mass per skipped (h,n,c)

LAST_RESULT = None         # BassKernelResults of the most recent run (for test.py)


def _bf16(a):
    import ml_dtypes
    return np.asarray(a, dtype=np.float32).astype(ml_dtypes.bfloat16)


def _host_prep(attention_centers, attention_spreads, value_w):
    """u -> stabilized exp tables, 1/Z, chunk keep-sets, strip crops."""
    ac = np.asarray(attention_centers, dtype=np.float32)
    sp = np.asarray(attention_spreads, dtype=np.float32)
    inv_cov = np.einsum("hij,hkj->hik", sp, sp).astype(np.float32)
    a, bb, c = inv_cov[:, 0, 0], inv_cov[:, 0, 1], inv_cov[:, 1, 1]
    mu1, mu2 = ac[:, 0], ac[:, 1]
    u1 = a * mu1 + bb * mu2
    u2 = c * mu2 + bb * mu1
    u3 = -0.5 * a
    u4 = -0.5 * c
    u5 = -bb

    # tab[h, 63*X + B] = exp(score(dx=31-X, dy=31-B) - max_h)
    dx = (31 - np.arange(63, dtype=np.float32))[:, None]
    dy = (31 - np.arange(63, dtype=np.float32))[None, :]
    sc = (u1[:, None, None] * dx + u2[:, None, None] * dy
          + u3[:, None, None] * dx * dx + u4[:, None, None] * dy * dy
          + u5[:, None, None] * dx * dy).astype(np.float32)
    sc -= sc.max(axis=(1, 2), keepdims=True)
    tab2d = np.exp(sc.astype(np.float64))                      # [9, 63, 63]
    tab_bf = _bf16(tab2d)                                      # device dtype

    # Z[h, iq, jq] = sum over the 32x32 window of the bf16-rounded table so
    # the normalization matches what the PE actually accumulates
    cs = np.pad(tab_bf.astype(np.float64).cumsum(1).cumsum(2),
                ((0, 0), (1, 0), (1, 0)))
    i0 = np.arange(32)
    zi, zj = np.meshgrid(i0, i0, indexing="ij")
    z = (cs[:, zi + 32, zj + 32] - cs[:, zi, zj + 32]
         - cs[:, zi + 32, zj] + cs[:, zi, zj])                 # [9, 32, 32]
    rz = (1.0 / z).reshape(NH, S)

    # keep[h][n][c]: does kl-chunk c carry non-negligible softmax mass for
    # any output in half n?  Computed exactly from the strip index formula:
    # chunk (h,n,c) reads tabflat[offp + o + 63*fi + fj], o=1764+1008n-252c.
    tabflat = tab_bf.astype(np.float64).reshape(NH, TBL)
    offp = (63 * (np.arange(128) // 32) + np.arange(128) % 32)
    fi = np.arange(16)
    fj = np.arange(32)
    idx0 = (offp[:, None, None] + 63 * fi[None, :, None]
            + fj[None, None, :])                               # [128,16,32]
    keep = np.zeros((NH, 2, 8), dtype=bool)
    dropped = np.zeros((NH, 32, 32))
    for h in range(NH):
        zh = z[h]
        for n in range(2):
            for cc in range(8):
                o = 1764 + 1008 * n - 252 * cc
                mass = tabflat[h][idx0 + o].sum(axis=0)        # [16,32]
                rel = mass / zh[16 * n:16 * n + 16, :]
                if rel.max() >= SKIP_THR:
                    keep[h, n, cc] = True
                else:
                    dropped[h, 16 * n:16 * n + 16, :] += rel
    assert dropped.max() < 2e-3, f"dropped softmax mass {dropped.max():.2e}"

    # strip crop per head over kept (n, c)
    lo = np.zeros(NH, dtype=int)
    width = np.zeros(NH, dtype=int)
    for h in range(NH):
        os_ = [1764 + 1008 * n - 252 * cc
               for n in range(2) for cc in range(8) if keep[h, n, cc]]
        lo[h] = min(os_)
        width[h] = max(os_) + 63 * 15 + 31 - lo[h] + 1

    vw = np.asarray(value_w, dtype=np.float32)                 # [256, 2304]
    wt = np.ascontiguousarray(
        vw.reshape(D, NH, 2, 128).transpose(3, 1, 2, 0).reshape(128, NH * 2, D))
    return (tab_bf.reshape(NH, TBL).copy(), _bf16(rz), _bf16(wt),
            keep, lo, width)


def _build_program(keep, lo, width):
    nc = bacc.Bacc("TRN2", target_bir_lowering=False, debug=False)
    x_d = nc.declare_dram_parameter("x", [128, BLOC, 8, D], BF16, isOutput=False)
    wt_d = nc.declare_dram_parameter("wt", [128, NH * 2, D], BF16, isOutput=False)
    tab_d = nc.declare_dram_parameter("tab", [NH, TBL], BF16, isOutput=False)
    rz_d = nc.declare_dram_parameter("rz", [NH, S], BF16, isOutput=False)
    vb_d = nc.declare_dram_parameter("vb", [D], F32, isOutput=False)
    y_d = nc.declare_dram_parameter("y", [BLOC, D, S], BF16, isOutput=True)

    with TileContext(nc) as tc:
        with tc.tile_pool(name="singles", bufs=1) as singles, \
             tc.tile_pool(name="vs", bufs=1) as vpool, \
             tc.tile_pool(name="outs", bufs=4) as opool, \
             tc.tile_pool(name="pa", bufs=1, space="PSUM") as pa:

            # spread the load prefix over all 3 DMA-capable queues
            # (SP + Activation HWDGE, GPSIMD SWDGE), ordered by first use:
            # x[b0] + strips 0-2, then rz 0-2, strips 3-5, ... wt last.
            x_sb = [singles.tile([128, 8, D], BF16, tag=f"x{bb}",
                                 name=f"x{bb}") for bb in range(BLOC)]
            nc.sync.dma_start(out=x_sb[0], in_=x_d[:, 0])
            strip = {}
            rz_t = {}
            for h in range(NH):
                strip[h] = singles.tile([128, int(width[h])], BF16,
                                        tag=f"strip{h}", name=f"strip{h}")
                rz_t[h] = singles.tile([128, S], BF16, tag=f"rz{h}",
                                       name=f"rz{h}")
            wt_sb = singles.tile([128, NH * 2, D], BF16)
            vb_sb = singles.tile([128, 2], F32)

            def load_strip(e, h):
                e.dma_start(
                    out=strip[h],
                    in_=bass.AP(tensor=tab_d, offset=h * TBL + int(lo[h]),
                                ap=[[63, 4], [1, 32], [1, int(width[h])]]))

            def load_rz(e, h):
                e.dma_start(
                    out=rz_t[h],
                    in_=bass.AP(tensor=rz_d, offset=h * S,
                                ap=[[0, 128], [1, S]]))

            load_strip(nc.scalar, 0)
            load_strip(nc.gpsimd, 1)
            load_strip(nc.sync, 2)
            nc.scalar.dma_start(out=wt_sb, in_=wt_d[:, :, :])
            load_rz(nc.gpsimd, 0)
            load_strip(nc.sync, 3)
            load_rz(nc.gpsimd, 1)
            load_strip(nc.scalar, 4)
            load_rz(nc.sync, 2)
            load_strip(nc.gpsimd, 5)
            load_rz(nc.scalar, 3)
            load_strip(nc.sync, 6)
            load_rz(nc.gpsimd, 4)
            load_strip(nc.scalar, 7)
            load_rz(nc.sync, 5)
            load_strip(nc.gpsimd, 8)
            load_rz(nc.scalar, 6)
            load_rz(nc.sync, 7)
            load_rz(nc.gpsimd, 8)
            nc.sync.dma_start(out=x_sb[1], in_=x_d[:, 1])
            nc.gpsimd.dma_start(
                out=vb_sb, in_=bass.AP(tensor=vb_d, offset=0,
                                       ap=[[1, 128], [128, 2]]))

            for b in range(BLOC):
                vt = {}
                for h in range(NH):       # strip h first needed ~9us apart
                    for m in range(2):    # din chunk
                        pair = (2 * h + m) % 3
                        ps = {}
                        for n in range(2):
                            cs_ = [cc for cc in range(8) if keep[h, n, cc]]
                            ps[n] = pa.tile([128, 512], F32,
                                            tag=f"bank{2 * pair + n}",
                                            name=f"bank{2 * pair + n}")
                            for cc in cs_:
                                s_t = strip[h]
                                o = 1764 + 1008 * n - 252 * cc - int(lo[h])
                                rhs = bass.AP(
                                    tensor=s_t.tensor,
                                    offset=s_t.offset + o,
                                    ap=[s_t.ap[0], [63, 16], [1, 32]])
                                nc.tensor.matmul(
                                    ps[n],
                                    lhsT=x_sb[b][:, cc,
                                                 m * 128:(m + 1) * 128],
                                    rhs=rhs,
                                    start=(cc == cs_[0]),
                                    stop=(cc == cs_[-1]))
                        for n in range(2):
                            v = vpool.tile([128, 512], BF16,
                                           tag=f"v{2 * h + m}_{n}",
                                           name=f"v{2 * h + m}_{n}")
                            nc.vector.tensor_mul(
                                v, ps[n],
                                rz_t[h][:, 512 * n:512 * (n + 1)])
                            vt[(2 * h + m, n)] = v
                # stage B: out^T[dout, ij] += W^T chunk @ V, one accumulator
                # at a time so the evac + y write of each overlaps the next
                for n in range(2):
                    for do in range(2):
                        po = pa.tile([128, 512], F32,
                                     tag=f"bank{6 + (2 * n + do) % 2}",
                                     name=f"bank{6 + (2 * n + do) % 2}")
                        for q in range(NH * 2):
                            nc.tensor.matmul(
                                po,
                                lhsT=wt_sb[:, q, do * 128:(do + 1) * 128],
                                rhs=vt[(q, n)],
                                start=(q == 0), stop=(q == NH * 2 - 1))
                        ot = opool.tile([128, 512], BF16, tag="ot", name="ot")
                        nc.vector.tensor_scalar_add(ot, po, vb_sb[:, do:do + 1])
                        (nc.sync if (2 * n + do) % 2 == 0
                         else nc.scalar).dma_start(
                            out=y_d[b, do * 128:(do + 1) * 128,
                                    n * 512:(n + 1) * 512],
                            in_=ot)
    nc.compile()
    return nc


def kernel(hidden_states, attention_mask, attention_centers, attention_spreads,
           value_w, value_b, **_ignored):
    global LAST_RESULT
    hs = np.asarray(hidden_states, dtype=np.float32)
    tab, rz, wt, keep, lo, width = _host_prep(
        attention_centers, attention_spreads, value_w)
    vb = np.ascontiguousarray(np.asarray(value_b, dtype=np.float32))

    # per-core x: reverse kl within each 128-chunk, partition-major layout
    xr = hs.reshape(B, 8, 128, D)[:, :, ::-1, :]
    in_maps = []
    for cid in range(NCORES):
        xc = _bf16(np.ascontiguousarray(
            xr[cid * BLOC:(cid + 1) * BLOC].transpose(2, 0, 1, 3)))
        in_maps.append({"x": xc, "wt": wt, "tab": tab, "rz": rz, "vb": vb})

    nc = _build_program(keep, lo, width)
    LAST_RESULT = run_bass_kernel_spmd(nc, in_maps, core_ids=list(range(NCORES)))

    out = np.concatenate(
        [np.asarray(r["y"]).astype(np.float32).transpose(0, 2, 1)
         for r in LAST_RESULT.results], axis=0)
    return np.ascontiguousarray(out).reshape(B, W_IMG, H_IMG, D)
